# revision 31
# baseline (speedup 1.0000x reference)
"""Trainium2 Bass kernel for a 2-layer GRU (Keras reset_after) + 3 Dense layers.

Model (per reference):
  h1 = GRU(x; k1, r1, b1)            # [B,T,64] -> [B,T,256], full sequence
  h2 = GRU(h1; k2, r2, b2)[:, -1]    # last state, [B,128]
  y  = ((h2 @ w3 + b3) @ w4 + b4) @ w5 + b5   # [B,24]

Strategy: pure data parallel over 8 NeuronCores (batch 256 -> 32 per core).
Everything on-chip per core; transposed layout (units on partitions, batch on
the free dim) so the sequential scan needs no per-step transposes.

Critical-path-oriented schedule (v2):
  - GRU1 z/r gate columns reordered [r | z] so the reset gate's matmuls
    finish first; one sigmoid covers both (z negated so sigma gives w=1-z).
  - State update uses h' = g + w*hh with g = h - w*h precomputed on the
    GpSimd engine off the critical chain (only e = w*hh and h' = g + e
    remain on it).
  - GRU2 runs one step behind GRU1; its off-chain elementwise work
    (f2/g2/e2/h2c) lives on GpSimd, keeping DVE/ACT clear for GRU1's chain.
  - GRU2's recurrent-h matmul (rh2) is ordered before xh2 so the reset
    multiply can start right after sigma2.

All matmul weights/activations fp16 (fp32 PSUM accumulate), gate math fp32
internally on DVE/ACT with fp16 storage.
"""

import numpy as np

import concourse.bass as bass
import concourse.mybir as mybir
import concourse.tile as tile
from concourse import bacc
from concourse.bass_utils import run_bass_kernel_spmd
from concourse.tile import add_dep_helper

F16 = mybir.dt.float16
F32 = mybir.dt.float32
AF = mybir.ActivationFunctionType
OP = mybir.AluOpType

B, T_FULL, F = 256, 512, 64
U1, U2, OUT = 256, 128, 24
NCORES = 8
BL = B // NCORES  # 32 local batch


def _prep(inputs, T):
    """Host-side preprocessing -> (list of per-core input dicts, flags)."""
    x = np.asarray(inputs["x"], np.float32)[:, :T, :]
    k1 = np.asarray(inputs["k1"], np.float32)
    r1 = np.asarray(inputs["r1"], np.float32)
    b1 = np.asarray(inputs["b1"], np.float32)
    k2 = np.asarray(inputs["k2"], np.float32)
    r2 = np.asarray(inputs["r2"], np.float32)
    b2 = np.asarray(inputs["b2"], np.float32)
    w3 = np.asarray(inputs["w3"], np.float32)
    b3 = np.asarray(inputs["b3"], np.float32)
    w4 = np.asarray(inputs["w4"], np.float32)
    b4 = np.asarray(inputs["b4"], np.float32)
    w5 = np.asarray(inputs["w5"], np.float32)
    b5 = np.asarray(inputs["b5"], np.float32)

    # GRU1 z/r columns reordered to [r | -z] (reset first; z negated so
    # sigmoid yields w = 1 - z directly).
    def zr_reorder(m):  # m: [*, 3U1] -> [*, 2U1] with [r | -z]
        return np.concatenate([m[..., U1 : 2 * U1], -m[..., :U1]], -1)

    k1zr = zr_reorder(k1)
    b1zr = zr_reorder(b1[0] + b1[1])
    r1zr = zr_reorder(r1)

    s2 = np.ones(3 * U2, np.float32)
    s2[:U2] = -1.0
    k2e = k2 * s2
    r2e = r2 * s2
    # z/r slabs with an extra non-negated z copy: [z- | r | z+]
    b2s = b2[0] + b2[1]
    b2zr = np.concatenate([(b2s[: 2 * U2]) * s2[: 2 * U2], b2s[:U2]])  # [3*U2]

    # k1 z/r part augmented with bias row: [65, 512] cols [r|−z]
    wk1zr = np.concatenate([k1zr, b1zr[None, :]], 0)
    # k1 h part augmented with input-bias row: [65, 256]
    wk1h = np.concatenate([k1[:, 2 * U1 :], b1[0][None, 2 * U1 :]], 0)
    # r1 tiles: m-order [r0 r1 z0 z1 h0 h1], tile (m,k) at cols (m*2+k)*128
    r1n = np.concatenate([r1zr, r1[:, 2 * U1 :]], 1)  # [256, 1536]
    wr1 = r1n.reshape(2, 128, 6, 128).transpose(1, 2, 0, 3).reshape(128, 12 * 128)
    # k2 tiles: m-order [z- r z+ h], tile (m,k) at cols (m*2+k)*128
    k2n = np.concatenate([k2e[:, : 2 * U2], k2[:, :U2], k2[:, 2 * U2 :]], 1)
    wk2 = k2n.reshape(2, 128, 4, 128).transpose(1, 2, 0, 3).reshape(128, 8 * 128)
    # r2 slabs [z- r z+ h] at m*128
    wr2 = np.concatenate([r2e[:, : 2 * U2], r2[:, :U2], r2[:, 2 * U2 :]], 1)

    vb1h = np.stack([b1[1, 2 * U1 : 2 * U1 + 128], b1[1, 2 * U1 + 128 :]], 1)  # [128,2]
    vb2h = np.stack([b2[0, 2 * U2 :], b2[1, 2 * U2 :]], 1)  # [128,2]
    vbd = np.zeros((128, 3), np.float32)
    vbd[:64, 0] = b3
    vbd[:32, 1] = b4
    vbd[:OUT, 2] = b5

    flags = {
        "HAS_B1H": bool(np.any(b1[1, 2 * U1 :] != 0)),
        "HAS_B2ZR": bool(np.any(b2zr != 0)),
        "HAS_B2H": bool(np.any(b2[:, 2 * U2 :] != 0)),
    }

    # pack all fp16 weights into one [128, 4344] block (one DMA):
    # cols: wk1h 0:256 | wk1zr 256:768 | wr1 768:2304 | wk2 2304:3328 |
    #       wr2 3328:3840 | wb2zr 3840:4224 (row 0) | wd3 4224:4288 |
    #       wd4 4288:4320 | wd5 4320:4344
    pack = np.zeros((128, 4344), np.float32)
    pack[: F + 1, 0:256] = wk1h
    pack[: F + 1, 256:768] = wk1zr
    pack[:, 768:2304] = wr1
    pack[:, 2304:3328] = wk2
    pack[:, 3328:3840] = wr2
    pack[0, 3840 : 3840 + 384] = b2zr
    pack[:, 4224:4288] = w3
    pack[:64, 4288:4320] = w4
    pack[:32, 4320:4344] = w5
    shared = {
        "wpack": pack.astype(np.float16),
        "vb1h": vb1h.astype(np.float32),
        "vb2h": vb2h.astype(np.float32),
        "vbd": vbd.astype(np.float32),
    }

    in_maps = []
    for c in range(NCORES):
        xs = x[c * BL : (c + 1) * BL]  # [BL, T, F]
        xt = np.ascontiguousarray(xs.transpose(2, 1, 0)).reshape(F, T * BL)
        xin = np.concatenate([xt, np.ones((1, T * BL), np.float32)], 0)
        m = dict(shared)
        m["xin"] = xin.astype(np.float16)
        in_maps.append(m)
    return in_maps, flags


def _build(T, flags):
    """Emit the Bass program for T timesteps. Returns compiled nc."""
    HAS_B1H = flags["HAS_B1H"]
    HAS_B2ZR = flags["HAS_B2ZR"]
    HAS_B2H = flags["HAS_B2H"]
    nc = bacc.Bacc("TRN2", target_bir_lowering=False, debug=False, num_devices=NCORES)

    d_xin = nc.dram_tensor("xin", [F + 1, T * BL], F16, kind="ExternalInput").ap()
    d_wpack = nc.dram_tensor("wpack", [128, 4344], F16, kind="ExternalInput").ap()
    d_vb1h = nc.dram_tensor("vb1h", [128, 2], F32, kind="ExternalInput").ap()
    d_vb2h = nc.dram_tensor("vb2h", [128, 2], F32, kind="ExternalInput").ap()
    d_vbd = nc.dram_tensor("vbd", [128, 3], F32, kind="ExternalInput").ap()
    d_y = nc.dram_tensor("y", [BL, OUT], F32, kind="ExternalOutput").ap()

    with tile.TileContext(nc) as tc:
        with (
            tc.tile_pool(name="big", bufs=1) as big,
            tc.tile_pool(name="wts", bufs=1) as wts,
            tc.tile_pool(name="state", bufs=1) as state,
            tc.tile_pool(name="tmp", bufs=3) as tmp,
        ):
            sb_x = big.tile([F + 1, T * BL], F16, tag="sb_x", name="sb_x")
            sb_xg1h = big.tile([128, T, 64], F16, tag="sb_xg1h", name="sb_xg1h")

            sb_wpack = wts.tile([128, 4344], F16, tag="sb_wpack", name="sb_wpack")
            # wk1h slab first (the bulk phase needs only it + x chunk 0)
            nc.sync.dma_start(
                out=sb_wpack[: F + 1, 0:256], in_=d_wpack[: F + 1, 0:256]
            )

            # x load, split for early bulk start
            nchunk = 8
            cw = (T * BL) // nchunk
            for i in range(nchunk):
                nc.sync.dma_start(
                    out=sb_x[:, i * cw : (i + 1) * cw],
                    in_=d_xin[:, i * cw : (i + 1) * cw],
                )

            # the rest of the weights in one DMA
            nc.sync.dma_start(out=sb_wpack[:, 256:], in_=d_wpack[:, 256:])

            def wtile(name, shape, dt, src):
                t_ = wts.tile(shape, dt, tag=name, name=name)
                nc.sync.dma_start(out=t_[:], in_=src[:])
                return t_

            sb_wk1h = sb_wpack[: F + 1, 0:256]
            sb_wk1zr = sb_wpack[: F + 1, 256:768]
            sb_wr1 = sb_wpack[:, 768:2304]
            sb_wk2 = sb_wpack[:, 2304:3328]
            sb_wr2 = sb_wpack[:, 3328:3840]
            sb_wb2zr = sb_wpack[0:1, 3840:4224]
            sb_wd3 = sb_wpack[:, 4224:4288]
            sb_wd4 = sb_wpack[:64, 4288:4320]
            sb_wd5 = sb_wpack[:32, 4320:4344]
            sb_vb1h = wtile("sb_vb1h", [128, 2], F32, d_vb1h)
            sb_vb2h = wtile("sb_vb2h", [128, 2], F32, d_vb2h)
            sb_vbd = wtile("sb_vbd", [128, 3], F32, d_vbd)

            sb_ones = wts.tile([1, BL], F16, tag="sb_ones", name="sb_ones")
            nc.vector.memset(sb_ones[:], 1.0)

            # ---- bulk precompute xg1h = [x;1] @ [k1_h; b1_0h]  -> sb_xg1h ----
            with tc.tile_pool(name="bulkps", bufs=4, space="PSUM") as bulkps:
                CH = 16  # timesteps per matmul (N = CH*BL = 512)
                for ci in range((T + CH - 1) // CH):
                    t0 = ci * CH
                    ts_ = min(CH, T - t0)
                    n = ts_ * BL
                    for m in range(2):
                        pb = bulkps.tile([128, 512], F32, tag="pb", name="pb")
                        nc.tensor.matmul(
                            pb[:, :n],
                            sb_wk1h[:, m * 128 : (m + 1) * 128],
                            sb_x[:, t0 * BL : t0 * BL + n],
                            start=True,
                            stop=True,
                        )
                        dst = sb_xg1h[:, t0 : t0 + ts_, m * 32 : (m + 1) * 32]
                        src = pb.rearrange("p (t b) -> p t b", b=BL)[:, :ts_, :]
                        if m == 0:
                            nc.vector.tensor_copy(dst, src)
                        else:
                            nc.scalar.copy(dst, src)

            # ---- the scan ----
            # Recurrent-matmul split: h' = g + e with g = h - w*h (Pool,
            # ready early) and e = w*hh (ready right after tanh). The next
            # step's z/r projections accumulate R@g and R@e separately, so
            # the serial loop runs e -> R@e -> sigma -> ... -> e without the
            # h' add or a full matmul wait on it. h' itself is still formed
            # (DVE) for the k2/rh projections, which are off-loop.
            with tc.tile_pool(name="ps", bufs=1, space="PSUM") as psp:
                ps_zr1 = [
                    psp.tile([128, 128], F32, tag=f"ps_zr1_{i}", name=f"ps_zr1_{i}")
                    for i in range(2)
                ]
                ps_h1 = [
                    psp.tile([128, 64], F32, tag=f"ps_h1_{i}", name=f"ps_h1_{i}")
                    for i in range(2)
                ]
                # [z2- 0:32 | r2 32:64 | z2+ 64:96 | xh2 96:128 | rh2 128:160]
                ps_g2 = [
                    psp.tile([128, 160], F32, tag=f"ps_g2_{i}", name=f"ps_g2_{i}")
                    for i in range(2)
                ]
                sb_h1 = [
                    state.tile([128, 64], F16, tag=f"sb_h1_{i}", name=f"sb_h1_{i}")
                    for i in range(2)
                ]
                sb_h2 = [
                    state.tile([128, BL], F16, tag=f"sb_h2_{i}", name=f"sb_h2_{i}")
                    for i in range(2)
                ]
                sb_w1 = [
                    state.tile([128, 128], F16, tag=f"sb_w1_{i}", name=f"sb_w1_{i}")
                    for i in range(2)
                ]
                sb_g1 = [
                    state.tile([128, 64], F16, tag=f"sb_g1_{i}", name=f"sb_g1_{i}")
                    for i in range(2)
                ]
                sb_e1 = [
                    state.tile([128, 64], F16, tag=f"sb_e1_{i}", name=f"sb_e1_{i}")
                    for i in range(2)
                ]
                sb_w2 = [
                    state.tile([128, 96], F16, tag=f"sb_w2_{i}", name=f"sb_w2_{i}")
                    for i in range(2)
                ]
                sb_g2 = [
                    state.tile([128, BL], F16, tag=f"sb_g2_{i}", name=f"sb_g2_{i}")
                    for i in range(2)
                ]
                sb_e2 = [
                    state.tile([128, BL], F16, tag=f"sb_e2_{i}", name=f"sb_e2_{i}")
                    for i in range(2)
                ]

                def emit_xg1(s):
                    """x-side z/r projections (+biases) for step s into ps_zr1[s%2].
                    First matmul start=True zeroes the PSUM bank."""
                    pzr = ps_zr1[s % 2]
                    rhs = sb_x[:, s * BL : (s + 1) * BL]
                    for m in range(4):
                        nc.tensor.matmul(
                            pzr[:, m * 32 : (m + 1) * 32],
                            sb_wk1zr[:, m * 128 : (m + 1) * 128],
                            rhs,
                            start=(m == 0),
                            stop=(s == 0),
                        )

                def emit_rh1(t):
                    """Recurrent h-part (rh1) for step t from full h1(t-1)."""
                    ph = ps_h1[t % 2]
                    h1p = sb_h1[(t - 1) % 2]
                    for i, m in enumerate((4, 5)):
                        for k in range(2):
                            nc.tensor.matmul(
                                ph[:, i * 32 : (i + 1) * 32],
                                sb_wr1[:, (m * 2 + k) * 128 : (m * 2 + k + 1) * 128],
                                h1p[:, k * 32 : (k + 1) * 32],
                                start=(i == 0 and k == 0),
                                stop=(k == 1),
                            )

                def emit_rzr1_part(t, rhs64, start_first, stop_last):
                    """z/r recurrent projections for step t+1 against one of
                    g1(t)/e1(t)/h1(t) (rhs64: [128, 64])."""
                    pzr = ps_zr1[(t + 1) % 2]
                    for m in range(4):
                        for k in range(2):
                            nc.tensor.matmul(
                                pzr[:, m * 32 : (m + 1) * 32],
                                sb_wr1[:, (m * 2 + k) * 128 : (m * 2 + k + 1) * 128],
                                rhs64[:, k * 32 : (k + 1) * 32],
                                start=(start_first and m == 0 and k == 0),
                                stop=(stop_last and k == 1),
                            )

                def emit_k2(s):
                    """GRU2 z-/r/z+ and xh2 input projections from h1(s)
                    (+bias). First matmul start=True zeroes the bank."""
                    pg2 = ps_g2[s % 2]
                    h1s = sb_h1[s % 2]
                    for m in range(3):  # z-/r/z+ gates
                        reg = pg2[:, m * 32 : (m + 1) * 32]
                        for k in range(2):
                            nc.tensor.matmul(
                                reg,
                                sb_wk2[:, (m * 2 + k) * 128 : (m * 2 + k + 1) * 128],
                                h1s[:, k * 32 : (k + 1) * 32],
                                start=(s <= 1 and m == 0 and k == 0),
                                stop=False,
                            )
                        if HAS_B2ZR:
                            nc.tensor.matmul(
                                reg,
                                sb_wb2zr[:, m * 128 : (m + 1) * 128],
                                sb_ones[:],
                                start=False,
                                stop=False,
                            )
                    for k in range(2):  # xg2 h-part
                        nc.tensor.matmul(
                            pg2[:, 96:128],
                            sb_wk2[:, (6 + k) * 128 : (7 + k) * 128],
                            h1s[:, k * 32 : (k + 1) * 32],
                            start=False,
                            stop=(k == 1),
                        )

                def emit_rh2(s):
                    """rh2 for gru2 step s from full h2(s-1)."""
                    nc.tensor.matmul(
                        ps_g2[s % 2][:, 128:160],
                        sb_wr2[:, 384:512],
                        sb_h2[(s - 1) % 2][:],
                        start=False,
                        stop=True,
                    )

                def emit_rzr2_part(s, rhs32, stop_last, start_first=False):
                    """z-/r/z+ recurrent projections for gru2 step s+1 against
                    one of g2(s)/e2(s)/h2(s) (rhs32: [128, 32])."""
                    pg2 = ps_g2[(s + 1) % 2]
                    for m in range(3):
                        nc.tensor.matmul(
                            pg2[:, m * 32 : (m + 1) * 32],
                            sb_wr2[:, m * 128 : (m + 1) * 128],
                            rhs32[:],
                            start=(start_first and m == 0),
                            stop=stop_last,
                        )

                def emit_gru1_elt_a(t, pre1):
                    """sigma(rz1) on ACT; t1/pre1 on DVE; f1/g1 on Pool.
                    Returns (sig1_inst, pre1_inst)."""
                    pzr, ph = ps_zr1[t % 2], ps_h1[t % 2]
                    wsb = sb_w1[t % 2]  # [r1s 0:64 | w1 64:128]
                    i_sig1 = nc.scalar.activation(wsb[:], pzr[:], AF.Sigmoid)
                    if t == 0:
                        return i_sig1, None
                    h1p = sb_h1[(t - 1) % 2]
                    t1 = tmp.tile([128, 64], F16, tag="t1", name="t1")
                    if HAS_B1H:
                        for i in range(2):
                            nc.vector.scalar_tensor_tensor(
                                t1[:, i * 32 : (i + 1) * 32],
                                ph[:, i * 32 : (i + 1) * 32],
                                sb_vb1h[:, i : i + 1],
                                wsb[:, i * 32 : 32 + i * 32],
                                OP.add,
                                OP.mult,
                            )
                    else:
                        nc.vector.tensor_mul(t1[:], ph[:], wsb[:, 0:64])
                    i_pre1 = nc.vector.tensor_add(pre1[:], t1[:], sb_xg1h[:, t, :])
                    # off-chain: g1 = h1p - w1*h1p on Pool
                    f1 = tmp.tile([128, 64], F16, tag="f1", name="f1")
                    g1 = sb_g1[t % 2]
                    nc.gpsimd.tensor_mul(f1[:], wsb[:, 64:128], h1p[:])
                    nc.gpsimd.tensor_sub(g1[:], h1p[:], f1[:])
                    return i_sig1, i_pre1

                def emit_gru1_elt_b(t, pre1):
                    """tanh on ACT; e1/h1c on DVE. Returns tanh1_inst."""
                    wsb = sb_w1[t % 2]
                    h1c = sb_h1[t % 2]
                    hh1 = tmp.tile([128, 64], F16, tag="hh1", name="hh1")
                    if t == 0:
                        i_tanh1 = nc.scalar.activation(
                            hh1[:], sb_xg1h[:, 0, :], AF.Tanh
                        )
                        nc.vector.tensor_mul(h1c[:], wsb[:, 64:128], hh1[:])
                        return i_tanh1, None
                    i_tanh1 = nc.scalar.activation(hh1[:], pre1[:], AF.Tanh)
                    e1 = sb_e1[t % 2]
                    i_e1 = nc.vector.tensor_mul(e1[:], wsb[:, 64:128], hh1[:])
                    nc.vector.tensor_add(h1c[:], sb_g1[t % 2][:], e1[:])
                    return i_tanh1, i_e1

                def emit_gru2_elt_a(s, t2b, i_sig1=None, i_pre1=None):
                    """sigma2 on ACT; t2a/t2b on DVE; g2 on Pool (z2+ dup
                    gives 1-w2 directly). Pinned behind gru1's sigma/pre1."""
                    pg2 = ps_g2[s % 2]
                    wr2sb = sb_w2[s % 2]  # [w2 0:32 | r2s 32:64 | 1-w2 64:96]
                    i_sig2 = nc.scalar.activation(wr2sb[:], pg2[:, 0:96], AF.Sigmoid)
                    if i_sig1 is not None:
                        add_dep_helper(i_sig2.ins, i_sig1.ins, reason="act order: sig1 first")
                    if s == 0:
                        nc.vector.tensor_scalar_add(
                            t2b[:], pg2[:, 96:128], sb_vb2h[:, 0:1]
                        )
                        return
                    h2p = sb_h2[(s - 1) % 2]
                    t2a = tmp.tile([128, BL], F16, tag="t2a", name="t2a")
                    if HAS_B2H:
                        i_t2a = nc.vector.scalar_tensor_tensor(
                            t2a[:],
                            pg2[:, 128:160],
                            sb_vb2h[:, 1:2],
                            wr2sb[:, 32:64],
                            OP.add,
                            OP.mult,
                        )
                        i_t2b = nc.vector.scalar_tensor_tensor(
                            t2b[:],
                            t2a[:],
                            sb_vb2h[:, 0:1],
                            pg2[:, 96:128],
                            OP.add,
                            OP.add,
                        )
                    else:
                        i_t2a = nc.vector.tensor_mul(
                            t2a[:], pg2[:, 128:160], wr2sb[:, 32:64]
                        )
                        i_t2b = nc.vector.tensor_add(t2b[:], t2a[:], pg2[:, 96:128])
                    if i_pre1 is not None:
                        add_dep_helper(i_t2a.ins, i_pre1.ins, reason="dve order: pre1 first")
                    g2 = sb_g2[s % 2]
                    nc.gpsimd.tensor_mul(g2[:], wr2sb[:, 64:96], h2p[:])
                    return i_t2b

                def emit_gru2_elt_b(s, t2b, i_tanh1=None):
                    """tanh2 on ACT; e2 on DVE; h2c on Pool."""
                    wr2sb = sb_w2[s % 2]
                    h2c = sb_h2[s % 2]
                    hh2 = tmp.tile([128, BL], F16, tag="hh2", name="hh2")
                    i_tanh2 = nc.scalar.activation(hh2[:], t2b[:], AF.Tanh)
                    if i_tanh1 is not None:
                        add_dep_helper(i_tanh2.ins, i_tanh1.ins, reason="act order: tanh1 first")
                    if s == 0:
                        nc.vector.tensor_mul(h2c[:], wr2sb[:, 0:32], hh2[:])
                        return
                    e2 = sb_e2[s % 2]
                    nc.vector.tensor_mul(e2[:], wr2sb[:, 0:32], hh2[:])
                    nc.gpsimd.tensor_add(h2c[:], sb_g2[s % 2][:], e2[:])

                # ---- schedule ----
                # Loop iteration t emits (steady state):
                #   PE : rh1(t)@h1, k2(t-1)+xh2(t-1)@h1, rh2(t-1)@h2,
                #        xg1(t+1), R1@g1(t), R2@g2(t-1), R1@e1(t), R2@e2(t-1)
                #   ACT: sigma_rz1(t), sigma2(t-1), tanh1(t), tanh2(t-1)
                #   DVE: t1, pre1, t2a, t2b, e1, h1c, e2
                #   Pool: f1, g1, f2, g2, h2c
                emit_xg1(0)
                for t in range(T):
                    pre1 = tmp.tile([128, 64], F16, tag="pre1", name="pre1")
                    t2b = tmp.tile([128, BL], F16, tag="t2b", name="t2b")
                    if t >= 2:
                        emit_rh2(t - 1)
                    if t + 1 < T:
                        emit_xg1(t + 1)
                    i_sig1, i_pre1 = emit_gru1_elt_a(t, pre1)
                    i_t2b = None
                    if t >= 1:
                        i_t2b = emit_gru2_elt_a(t - 1, t2b, i_sig1, i_pre1)
                    # R1 @ g1(t) (waits g1 from Pool, mid-step)
                    if t >= 1 and t + 1 < T:
                        emit_rzr1_part(t, sb_g1[t % 2], False, False)
                    i_tanh1, i_e1 = emit_gru1_elt_b(t, pre1)
                    if i_t2b is not None and i_e1 is not None:
                        add_dep_helper(i_t2b.ins, i_e1.ins, reason="dve order: e1 first")
                    if t >= 1:
                        emit_gru2_elt_b(t - 1, t2b, i_tanh1)
                    # R1 @ e1(t) (waits e1, late-step; closes gates(t+1) z/r)
                    if t + 1 < T:
                        emit_rzr1_part(
                            t, sb_h1[0] if t == 0 else sb_e1[t % 2], False, True
                        )
                    # R2 @ g2(t-1): first writer of gates2(t), zeroes the bank
                    if t >= 2:
                        emit_rzr2_part(
                            t - 1, sb_g2[(t - 1) % 2], False, start_first=True
                        )
                    if t + 1 < T:
                        emit_rh1(t + 1)
                    emit_k2(t)
                    # R2 @ e2(t-1)
                    if t >= 1:
                        emit_rzr2_part(
                            t - 1,
                            sb_h2[0] if t == 1 else sb_e2[(t - 1) % 2],
                            True,
                        )
                # gru2 tail for step T-1
                t2b = tmp.tile([128, BL], F16, tag="t2b", name="t2b")
                emit_rh2(T - 1)
                emit_gru2_elt_a(T - 1, t2b)
                emit_gru2_elt_b(T - 1, t2b)

                # ---- dense tail ----
                pd = ps_zr1[T % 2]
                h2f = sb_h2[(T - 1) % 2]
                q3 = tmp.tile([64, 32], F16, tag="q3", name="q3")
                q4 = tmp.tile([32, 32], F16, tag="q4", name="q4")
                q5 = tmp.tile([32, 32], F32, tag="q5", name="q5")
                qt = tmp.tile([32, 32], F32, tag="qt", name="qt")
                nc.vector.memset(q5[:], 0.0)
                nc.tensor.matmul(pd[0:64, 0:32], sb_wd3[:], h2f[:], start=True, stop=True)
                nc.scalar.activation(
                    q3[:], pd[0:64, 0:32], AF.Identity, bias=sb_vbd[0:64, 0:1]
                )
                nc.tensor.matmul(pd[0:32, 32:64], sb_wd4[:], q3[:], start=False, stop=True)
                nc.scalar.activation(
                    q4[:], pd[0:32, 32:64], AF.Identity, bias=sb_vbd[0:32, 1:2]
                )
                nc.tensor.matmul(pd[0:OUT, 64:96], sb_wd5[:], q4[:], start=False, stop=True)
                nc.scalar.activation(
                    q5[0:OUT, :], pd[0:OUT, 64:96], AF.Identity, bias=sb_vbd[0:OUT, 2:3]
                )
                nc.vector.transpose(qt[:], q5[:])
                nc.sync.dma_start(out=d_y[:], in_=qt[0:BL, 0:OUT])

    nc.compile()
    return nc


def _run(inputs, T):
    in_maps, flags = _prep(inputs, T)
    nc = _build(T, flags)
    res = run_bass_kernel_spmd(nc, in_maps, core_ids=list(range(NCORES)))
    return np.concatenate([res.results[c]["y"] for c in range(NCORES)], 0).astype(
        np.float32
    )


def kernel(**inputs):
    return _run(inputs, T_FULL)


if __name__ == "__main__":
    rng = np.random.default_rng(0)
    ins = {
        "x": rng.standard_normal((B, T_FULL, F), np.float32),
        "k1": rng.standard_normal((F, 3 * U1), np.float32) * 0.05,
        "r1": rng.standard_normal((U1, 3 * U1), np.float32) * 0.05,
        "b1": np.zeros((2, 3 * U1), np.float32),
        "k2": rng.standard_normal((U1, 3 * U2), np.float32) * 0.05,
        "r2": rng.standard_normal((U2, 3 * U2), np.float32) * 0.05,
        "b2": np.zeros((2, 3 * U2), np.float32),
        "w3": rng.standard_normal((U2, 64), np.float32) * 0.05,
        "b3": np.zeros((64,), np.float32),
        "w4": rng.standard_normal((64, 32), np.float32) * 0.05,
        "b4": np.zeros((32,), np.float32),
        "w5": rng.standard_normal((32, OUT), np.float32) * 0.05,
        "b5": np.zeros((OUT,), np.float32),
    }
    y = _run(ins, 8)
    print("ran", y.shape, y[:2, :4])


# revision 32
# speedup vs baseline: 1.0161x; 1.0161x over previous
"""Trainium2 Bass kernel for a 2-layer GRU (Keras reset_after) + 3 Dense layers.

Model (per reference):
  h1 = GRU(x; k1, r1, b1)            # [B,T,64] -> [B,T,256], full sequence
  h2 = GRU(h1; k2, r2, b2)[:, -1]    # last state, [B,128]
  y  = ((h2 @ w3 + b3) @ w4 + b4) @ w5 + b5   # [B,24]

Strategy: pure data parallel over 8 NeuronCores (batch 256 -> 32 per core).
Everything on-chip per core; transposed layout (units on partitions, batch on
the free dim) so the sequential scan needs no per-step transposes.

Critical-path-oriented schedule (v2):
  - GRU1 z/r gate columns reordered [r | z] so the reset gate's matmuls
    finish first; one sigmoid covers both (z negated so sigma gives w=1-z).
  - State update uses h' = g + w*hh with g = h - w*h precomputed on the
    GpSimd engine off the critical chain (only e = w*hh and h' = g + e
    remain on it).
  - GRU2 runs one step behind GRU1; its off-chain elementwise work
    (f2/g2/e2/h2c) lives on GpSimd, keeping DVE/ACT clear for GRU1's chain.
  - GRU2's recurrent-h matmul (rh2) is ordered before xh2 so the reset
    multiply can start right after sigma2.

All matmul weights/activations fp16 (fp32 PSUM accumulate), gate math fp32
internally on DVE/ACT with fp16 storage.
"""

import numpy as np

import concourse.bass as bass
import concourse.mybir as mybir
import concourse.tile as tile
from concourse import bacc
from concourse.bass_utils import run_bass_kernel_spmd
from concourse.tile import add_dep_helper

F16 = mybir.dt.float16
F32 = mybir.dt.float32
AF = mybir.ActivationFunctionType
OP = mybir.AluOpType

B, T_FULL, F = 256, 512, 64
U1, U2, OUT = 256, 128, 24
NCORES = 8
BL = B // NCORES  # 32 local batch


def _prep(inputs, T):
    """Host-side preprocessing -> (list of per-core input dicts, flags)."""
    x = np.asarray(inputs["x"], np.float32)[:, :T, :]
    k1 = np.asarray(inputs["k1"], np.float32)
    r1 = np.asarray(inputs["r1"], np.float32)
    b1 = np.asarray(inputs["b1"], np.float32)
    k2 = np.asarray(inputs["k2"], np.float32)
    r2 = np.asarray(inputs["r2"], np.float32)
    b2 = np.asarray(inputs["b2"], np.float32)
    w3 = np.asarray(inputs["w3"], np.float32)
    b3 = np.asarray(inputs["b3"], np.float32)
    w4 = np.asarray(inputs["w4"], np.float32)
    b4 = np.asarray(inputs["b4"], np.float32)
    w5 = np.asarray(inputs["w5"], np.float32)
    b5 = np.asarray(inputs["b5"], np.float32)

    # GRU1 z/r columns reordered to [r | -z] (reset first; z negated so
    # sigmoid yields w = 1 - z directly).
    def zr_reorder(m):  # m: [*, 3U1] -> [*, 2U1] with [r | -z]
        return np.concatenate([m[..., U1 : 2 * U1], -m[..., :U1]], -1)

    k1zr = zr_reorder(k1)
    b1zr = zr_reorder(b1[0] + b1[1])
    r1zr = zr_reorder(r1)

    s2 = np.ones(3 * U2, np.float32)
    s2[:U2] = -1.0
    k2e = k2 * s2
    r2e = r2 * s2
    # z/r slabs with an extra non-negated z copy: [z- | r | z+]
    b2s = b2[0] + b2[1]
    b2zr = np.concatenate([(b2s[: 2 * U2]) * s2[: 2 * U2], b2s[:U2]])  # [3*U2]

    # k1 z/r part augmented with bias row: [65, 512] cols [r|−z]
    wk1zr = np.concatenate([k1zr, b1zr[None, :]], 0)
    # k1 h part augmented with input-bias row: [65, 256]
    wk1h = np.concatenate([k1[:, 2 * U1 :], b1[0][None, 2 * U1 :]], 0)
    # r1 tiles: m-order [r0 r1 z0 z1 h0 h1], tile (m,k) at cols (m*2+k)*128
    r1n = np.concatenate([r1zr, r1[:, 2 * U1 :]], 1)  # [256, 1536]
    wr1 = r1n.reshape(2, 128, 6, 128).transpose(1, 2, 0, 3).reshape(128, 12 * 128)
    # k2 tiles: m-order [z- r z+ h], tile (m,k) at cols (m*2+k)*128
    k2n = np.concatenate([k2e[:, : 2 * U2], k2[:, :U2], k2[:, 2 * U2 :]], 1)
    wk2 = k2n.reshape(2, 128, 4, 128).transpose(1, 2, 0, 3).reshape(128, 8 * 128)
    # r2 slabs [z- r z+ h] at m*128
    wr2 = np.concatenate([r2e[:, : 2 * U2], r2[:, :U2], r2[:, 2 * U2 :]], 1)

    vb1h = np.stack([b1[1, 2 * U1 : 2 * U1 + 128], b1[1, 2 * U1 + 128 :]], 1)  # [128,2]
    vb2h = np.stack([b2[0, 2 * U2 :], b2[1, 2 * U2 :]], 1)  # [128,2]
    vbd = np.zeros((128, 3), np.float32)
    vbd[:64, 0] = b3
    vbd[:32, 1] = b4
    vbd[:OUT, 2] = b5

    flags = {
        "HAS_B1H": bool(np.any(b1[1, 2 * U1 :] != 0)),
        "HAS_B2ZR": bool(np.any(b2zr != 0)),
        "HAS_B2H": bool(np.any(b2[:, 2 * U2 :] != 0)),
    }

    # pack all fp16 weights into one [128, 4344] block (one DMA):
    # cols: wk1h 0:256 | wk1zr 256:768 | wr1 768:2304 | wk2 2304:3328 |
    #       wr2 3328:3840 | wb2zr 3840:4224 (row 0) | wd3 4224:4288 |
    #       wd4 4288:4320 | wd5 4320:4344
    pack = np.zeros((128, 4344), np.float32)
    pack[: F + 1, 0:256] = wk1h
    pack[: F + 1, 256:768] = wk1zr
    pack[:, 768:2304] = wr1
    pack[:, 2304:3328] = wk2
    pack[:, 3328:3840] = wr2
    pack[0, 3840 : 3840 + 384] = b2zr
    pack[:, 4224:4288] = w3
    pack[:64, 4288:4320] = w4
    pack[:32, 4320:4344] = w5
    shared = {
        "wpack": pack.astype(np.float16),
        "vb1h": vb1h.astype(np.float32),
        "vb2h": vb2h.astype(np.float32),
        "vbd": vbd.astype(np.float32),
    }

    in_maps = []
    for c in range(NCORES):
        xs = x[c * BL : (c + 1) * BL]  # [BL, T, F]
        xt = np.ascontiguousarray(xs.transpose(2, 1, 0)).reshape(F, T * BL)
        xin = np.concatenate([xt, np.ones((1, T * BL), np.float32)], 0)
        m = dict(shared)
        m["xin"] = xin.astype(np.float16)
        in_maps.append(m)
    return in_maps, flags


def _build(T, flags):
    """Emit the Bass program for T timesteps. Returns compiled nc."""
    HAS_B1H = flags["HAS_B1H"]
    HAS_B2ZR = flags["HAS_B2ZR"]
    HAS_B2H = flags["HAS_B2H"]
    nc = bacc.Bacc("TRN2", target_bir_lowering=False, debug=False, num_devices=NCORES)

    d_xin = nc.dram_tensor("xin", [F + 1, T * BL], F16, kind="ExternalInput").ap()
    d_wpack = nc.dram_tensor("wpack", [128, 4344], F16, kind="ExternalInput").ap()
    d_vb1h = nc.dram_tensor("vb1h", [128, 2], F32, kind="ExternalInput").ap()
    d_vb2h = nc.dram_tensor("vb2h", [128, 2], F32, kind="ExternalInput").ap()
    d_vbd = nc.dram_tensor("vbd", [128, 3], F32, kind="ExternalInput").ap()
    d_y = nc.dram_tensor("y", [BL, OUT], F32, kind="ExternalOutput").ap()

    with tile.TileContext(nc) as tc:
        with (
            tc.tile_pool(name="big", bufs=1) as big,
            tc.tile_pool(name="wts", bufs=1) as wts,
            tc.tile_pool(name="state", bufs=1) as state,
            tc.tile_pool(name="tmp", bufs=3) as tmp,
        ):
            sb_x = big.tile([F + 1, T * BL], F16, tag="sb_x", name="sb_x")
            sb_xg1h = big.tile([128, T, 64], F16, tag="sb_xg1h", name="sb_xg1h")

            sb_wpack = wts.tile([128, 4344], F16, tag="sb_wpack", name="sb_wpack")
            # wk1h slab first (the bulk phase needs only it + x chunk 0)
            nc.sync.dma_start(
                out=sb_wpack[: F + 1, 0:256], in_=d_wpack[: F + 1, 0:256]
            )

            # x load, split for early bulk start
            nchunk = 8
            cw = (T * BL) // nchunk
            for i in range(nchunk):
                nc.sync.dma_start(
                    out=sb_x[:, i * cw : (i + 1) * cw],
                    in_=d_xin[:, i * cw : (i + 1) * cw],
                )

            # the rest of the weights in one DMA
            nc.sync.dma_start(out=sb_wpack[:, 256:], in_=d_wpack[:, 256:])

            def wtile(name, shape, dt, src):
                t_ = wts.tile(shape, dt, tag=name, name=name)
                nc.sync.dma_start(out=t_[:], in_=src[:])
                return t_

            sb_wk1h = sb_wpack[: F + 1, 0:256]
            sb_wk1zr = sb_wpack[: F + 1, 256:768]
            sb_wr1 = sb_wpack[:, 768:2304]
            sb_wk2 = sb_wpack[:, 2304:3328]
            sb_wr2 = sb_wpack[:, 3328:3840]
            sb_wb2zr = sb_wpack[0:1, 3840:4224]
            sb_wd3 = sb_wpack[:, 4224:4288]
            sb_wd4 = sb_wpack[:64, 4288:4320]
            sb_wd5 = sb_wpack[:32, 4320:4344]
            sb_vb1h = wtile("sb_vb1h", [128, 2], F32, d_vb1h)
            sb_vb2h = wtile("sb_vb2h", [128, 2], F32, d_vb2h)
            sb_vbd = wtile("sb_vbd", [128, 3], F32, d_vbd)

            sb_ones = wts.tile([1, BL], F16, tag="sb_ones", name="sb_ones")
            nc.vector.memset(sb_ones[:], 1.0)

            # ---- bulk precompute xg1h = [x;1] @ [k1_h; b1_0h]  -> sb_xg1h ----
            with tc.tile_pool(name="bulkps", bufs=4, space="PSUM") as bulkps:
                CH = 16  # timesteps per matmul (N = CH*BL = 512)
                for ci in range((T + CH - 1) // CH):
                    t0 = ci * CH
                    ts_ = min(CH, T - t0)
                    n = ts_ * BL
                    for m in range(2):
                        pb = bulkps.tile([128, 512], F32, tag="pb", name="pb")
                        nc.tensor.matmul(
                            pb[:, :n],
                            sb_wk1h[:, m * 128 : (m + 1) * 128],
                            sb_x[:, t0 * BL : t0 * BL + n],
                            start=True,
                            stop=True,
                        )
                        dst = sb_xg1h[:, t0 : t0 + ts_, m * 32 : (m + 1) * 32]
                        src = pb.rearrange("p (t b) -> p t b", b=BL)[:, :ts_, :]
                        if m == 0:
                            nc.vector.tensor_copy(dst, src)
                        else:
                            nc.scalar.copy(dst, src)

            # ---- the scan ----
            # Recurrent-matmul split: h' = g + e with g = h - w*h (Pool,
            # ready early) and e = w*hh (ready right after tanh). The next
            # step's z/r projections accumulate R@g and R@e separately, so
            # the serial loop runs e -> R@e -> sigma -> ... -> e without the
            # h' add or a full matmul wait on it. h' itself is still formed
            # (DVE) for the k2/rh projections, which are off-loop.
            with tc.tile_pool(name="ps", bufs=1, space="PSUM") as psp:
                ps_zr1 = [
                    psp.tile([128, 128], F32, tag=f"ps_zr1_{i}", name=f"ps_zr1_{i}")
                    for i in range(2)
                ]
                ps_h1 = [
                    psp.tile([128, 64], F32, tag=f"ps_h1_{i}", name=f"ps_h1_{i}")
                    for i in range(2)
                ]
                # [z2- 0:32 | r2 32:64 | z2+ 64:96 | xh2 96:128 | rh2 128:160]
                ps_g2 = [
                    psp.tile([128, 160], F32, tag=f"ps_g2_{i}", name=f"ps_g2_{i}")
                    for i in range(2)
                ]
                sb_h1 = [
                    state.tile([128, 64], F16, tag=f"sb_h1_{i}", name=f"sb_h1_{i}")
                    for i in range(2)
                ]
                sb_h2 = [
                    state.tile([128, BL], F16, tag=f"sb_h2_{i}", name=f"sb_h2_{i}")
                    for i in range(2)
                ]
                sb_w1 = [
                    state.tile([128, 128], F16, tag=f"sb_w1_{i}", name=f"sb_w1_{i}")
                    for i in range(2)
                ]
                sb_g1 = [
                    state.tile([128, 64], F16, tag=f"sb_g1_{i}", name=f"sb_g1_{i}")
                    for i in range(2)
                ]
                sb_e1 = [
                    state.tile([128, 64], F16, tag=f"sb_e1_{i}", name=f"sb_e1_{i}")
                    for i in range(2)
                ]
                sb_w2 = [
                    state.tile([128, 96], F16, tag=f"sb_w2_{i}", name=f"sb_w2_{i}")
                    for i in range(2)
                ]
                sb_g2 = [
                    state.tile([128, BL], F16, tag=f"sb_g2_{i}", name=f"sb_g2_{i}")
                    for i in range(2)
                ]
                sb_e2 = [
                    state.tile([128, BL], F16, tag=f"sb_e2_{i}", name=f"sb_e2_{i}")
                    for i in range(2)
                ]

                def emit_xg1(s):
                    """x-side z/r projections (+biases) for step s into ps_zr1[s%2].
                    First matmul start=True zeroes the PSUM bank."""
                    pzr = ps_zr1[s % 2]
                    rhs = sb_x[:, s * BL : (s + 1) * BL]
                    for m in range(4):
                        nc.tensor.matmul(
                            pzr[:, m * 32 : (m + 1) * 32],
                            sb_wk1zr[:, m * 128 : (m + 1) * 128],
                            rhs,
                            start=(m == 0),
                            stop=(s == 0),
                        )

                def emit_rh1(t):
                    """Recurrent h-part (rh1) for step t from full h1(t-1)."""
                    ph = ps_h1[t % 2]
                    h1p = sb_h1[(t - 1) % 2]
                    for i, m in enumerate((4, 5)):
                        for k in range(2):
                            nc.tensor.matmul(
                                ph[:, i * 32 : (i + 1) * 32],
                                sb_wr1[:, (m * 2 + k) * 128 : (m * 2 + k + 1) * 128],
                                h1p[:, k * 32 : (k + 1) * 32],
                                start=(i == 0 and k == 0),
                                stop=(k == 1),
                            )

                def emit_rzr1_part(t, rhs64, start_first, stop_last):
                    """z/r recurrent projections for step t+1 against one of
                    g1(t)/e1(t)/h1(t) (rhs64: [128, 64])."""
                    pzr = ps_zr1[(t + 1) % 2]
                    for m in range(4):
                        for k in range(2):
                            nc.tensor.matmul(
                                pzr[:, m * 32 : (m + 1) * 32],
                                sb_wr1[:, (m * 2 + k) * 128 : (m * 2 + k + 1) * 128],
                                rhs64[:, k * 32 : (k + 1) * 32],
                                start=(start_first and m == 0 and k == 0),
                                stop=(stop_last and k == 1),
                            )

                def emit_k2(s):
                    """GRU2 z-/r/z+ and xh2 input projections from h1(s)
                    (+bias). First matmul start=True zeroes the bank."""
                    pg2 = ps_g2[s % 2]
                    h1s = sb_h1[s % 2]
                    for m in range(3):  # z-/r/z+ gates
                        reg = pg2[:, m * 32 : (m + 1) * 32]
                        for k in range(2):
                            nc.tensor.matmul(
                                reg,
                                sb_wk2[:, (m * 2 + k) * 128 : (m * 2 + k + 1) * 128],
                                h1s[:, k * 32 : (k + 1) * 32],
                                start=(s <= 1 and m == 0 and k == 0),
                                stop=False,
                            )
                        if HAS_B2ZR:
                            nc.tensor.matmul(
                                reg,
                                sb_wb2zr[:, m * 128 : (m + 1) * 128],
                                sb_ones[:],
                                start=False,
                                stop=False,
                            )
                    for k in range(2):  # xg2 h-part
                        nc.tensor.matmul(
                            pg2[:, 96:128],
                            sb_wk2[:, (6 + k) * 128 : (7 + k) * 128],
                            h1s[:, k * 32 : (k + 1) * 32],
                            start=False,
                            stop=(k == 1),
                        )

                def emit_rh2(s):
                    """rh2 for gru2 step s from full h2(s-1)."""
                    nc.tensor.matmul(
                        ps_g2[s % 2][:, 128:160],
                        sb_wr2[:, 384:512],
                        sb_h2[(s - 1) % 2][:],
                        start=False,
                        stop=True,
                    )

                def emit_rzr2_part(s, rhs32, stop_last, start_first=False):
                    """z-/r/z+ recurrent projections for gru2 step s+1 against
                    one of g2(s)/e2(s)/h2(s) (rhs32: [128, 32])."""
                    pg2 = ps_g2[(s + 1) % 2]
                    for m in range(3):
                        nc.tensor.matmul(
                            pg2[:, m * 32 : (m + 1) * 32],
                            sb_wr2[:, m * 128 : (m + 1) * 128],
                            rhs32[:],
                            start=(start_first and m == 0),
                            stop=stop_last,
                        )

                def emit_gru1_elt_a(t, pre1):
                    """sigma(rz1) on ACT; t1/pre1 on DVE; f1/g1 on Pool.
                    Returns (sig1_inst, pre1_inst)."""
                    pzr, ph = ps_zr1[t % 2], ps_h1[t % 2]
                    wsb = sb_w1[t % 2]  # [r1s 0:64 | w1 64:128]
                    i_sig1 = nc.scalar.activation(wsb[:], pzr[:], AF.Sigmoid)
                    if t == 0:
                        return i_sig1, None
                    h1p = sb_h1[(t - 1) % 2]
                    t1 = tmp.tile([128, 64], F16, tag="t1", name="t1")
                    if HAS_B1H:
                        for i in range(2):
                            nc.vector.scalar_tensor_tensor(
                                t1[:, i * 32 : (i + 1) * 32],
                                ph[:, i * 32 : (i + 1) * 32],
                                sb_vb1h[:, i : i + 1],
                                wsb[:, i * 32 : 32 + i * 32],
                                OP.add,
                                OP.mult,
                            )
                    else:
                        nc.vector.tensor_mul(t1[:], ph[:], wsb[:, 0:64])
                    i_pre1 = nc.vector.tensor_add(pre1[:], t1[:], sb_xg1h[:, t, :])
                    # off-chain: g1 = h1p - w1*h1p on Pool
                    f1 = tmp.tile([128, 64], F16, tag="f1", name="f1")
                    g1 = sb_g1[t % 2]
                    nc.gpsimd.tensor_mul(f1[:], wsb[:, 64:128], h1p[:])
                    nc.gpsimd.tensor_sub(g1[:], h1p[:], f1[:])
                    return i_sig1, i_pre1

                def emit_gru1_elt_b(t, pre1):
                    """tanh on ACT; e1/h1c on DVE. Returns tanh1_inst."""
                    wsb = sb_w1[t % 2]
                    h1c = sb_h1[t % 2]
                    hh1 = tmp.tile([128, 64], F16, tag="hh1", name="hh1")
                    if t == 0:
                        i_tanh1 = nc.scalar.activation(
                            hh1[:], sb_xg1h[:, 0, :], AF.Tanh
                        )
                        nc.vector.tensor_mul(h1c[:], wsb[:, 64:128], hh1[:])
                        return i_tanh1
                    i_tanh1 = nc.scalar.activation(hh1[:], pre1[:], AF.Tanh)
                    e1 = sb_e1[t % 2]
                    nc.vector.tensor_mul(e1[:], wsb[:, 64:128], hh1[:])
                    nc.vector.tensor_add(h1c[:], sb_g1[t % 2][:], e1[:])
                    return i_tanh1

                def emit_gru2_elt_a(s, t2b, i_sig1=None, i_pre1=None):
                    """sigma2 on ACT; t2a/t2b on DVE; g2 on Pool (z2+ dup
                    gives 1-w2 directly). Pinned behind gru1's sigma/pre1."""
                    pg2 = ps_g2[s % 2]
                    wr2sb = sb_w2[s % 2]  # [w2 0:32 | r2s 32:64 | 1-w2 64:96]
                    i_sig2 = nc.scalar.activation(wr2sb[:], pg2[:, 0:96], AF.Sigmoid)
                    if i_sig1 is not None:
                        add_dep_helper(i_sig2.ins, i_sig1.ins, reason="act order: sig1 first")
                    if s == 0:
                        nc.vector.tensor_scalar_add(
                            t2b[:], pg2[:, 96:128], sb_vb2h[:, 0:1]
                        )
                        return
                    h2p = sb_h2[(s - 1) % 2]
                    t2a = tmp.tile([128, BL], F16, tag="t2a", name="t2a")
                    if HAS_B2H:
                        i_t2a = nc.vector.scalar_tensor_tensor(
                            t2a[:],
                            pg2[:, 128:160],
                            sb_vb2h[:, 1:2],
                            wr2sb[:, 32:64],
                            OP.add,
                            OP.mult,
                        )
                        nc.vector.scalar_tensor_tensor(
                            t2b[:],
                            t2a[:],
                            sb_vb2h[:, 0:1],
                            pg2[:, 96:128],
                            OP.add,
                            OP.add,
                        )
                    else:
                        i_t2a = nc.vector.tensor_mul(
                            t2a[:], pg2[:, 128:160], wr2sb[:, 32:64]
                        )
                        nc.vector.tensor_add(t2b[:], t2a[:], pg2[:, 96:128])
                    if i_pre1 is not None:
                        add_dep_helper(i_t2a.ins, i_pre1.ins, reason="dve order: pre1 first")
                    g2 = sb_g2[s % 2]
                    nc.gpsimd.tensor_mul(g2[:], wr2sb[:, 64:96], h2p[:])

                def emit_gru2_elt_b(s, t2b, i_tanh1=None):
                    """tanh2 on ACT; e2 on DVE; h2c on Pool."""
                    wr2sb = sb_w2[s % 2]
                    h2c = sb_h2[s % 2]
                    hh2 = tmp.tile([128, BL], F16, tag="hh2", name="hh2")
                    i_tanh2 = nc.scalar.activation(hh2[:], t2b[:], AF.Tanh)
                    if i_tanh1 is not None:
                        add_dep_helper(i_tanh2.ins, i_tanh1.ins, reason="act order: tanh1 first")
                    if s == 0:
                        nc.vector.tensor_mul(h2c[:], wr2sb[:, 0:32], hh2[:])
                        return
                    e2 = sb_e2[s % 2]
                    nc.vector.tensor_mul(e2[:], wr2sb[:, 0:32], hh2[:])
                    nc.gpsimd.tensor_add(h2c[:], sb_g2[s % 2][:], e2[:])

                # ---- schedule ----
                # Loop iteration t emits (steady state):
                #   PE : rh1(t)@h1, k2(t-1)+xh2(t-1)@h1, rh2(t-1)@h2,
                #        xg1(t+1), R1@g1(t), R2@g2(t-1), R1@e1(t), R2@e2(t-1)
                #   ACT: sigma_rz1(t), sigma2(t-1), tanh1(t), tanh2(t-1)
                #   DVE: t1, pre1, t2a, t2b, e1, h1c, e2
                #   Pool: f1, g1, f2, g2, h2c
                emit_xg1(0)
                for t in range(T):
                    pre1 = tmp.tile([128, 64], F16, tag="pre1", name="pre1")
                    t2b = tmp.tile([128, BL], F16, tag="t2b", name="t2b")
                    if t >= 2:
                        emit_rh2(t - 1)
                    if t + 1 < T:
                        emit_xg1(t + 1)
                    i_sig1, i_pre1 = emit_gru1_elt_a(t, pre1)
                    if t >= 1:
                        emit_gru2_elt_a(t - 1, t2b, i_sig1, i_pre1)
                    # R1 @ g1(t) (waits g1 from Pool, mid-step)
                    if t >= 1 and t + 1 < T:
                        emit_rzr1_part(t, sb_g1[t % 2], False, False)
                    i_tanh1 = emit_gru1_elt_b(t, pre1)
                    if t >= 1:
                        emit_gru2_elt_b(t - 1, t2b, i_tanh1)
                    # R1 @ e1(t) (waits e1, late-step; closes gates(t+1) z/r)
                    if t + 1 < T:
                        emit_rzr1_part(
                            t, sb_h1[0] if t == 0 else sb_e1[t % 2], False, True
                        )
                    # R2 @ g2(t-1): first writer of gates2(t), zeroes the bank
                    if t >= 2:
                        emit_rzr2_part(
                            t - 1, sb_g2[(t - 1) % 2], False, start_first=True
                        )
                    if t + 1 < T:
                        emit_rh1(t + 1)
                    emit_k2(t)
                    # R2 @ e2(t-1)
                    if t >= 1:
                        emit_rzr2_part(
                            t - 1,
                            sb_h2[0] if t == 1 else sb_e2[(t - 1) % 2],
                            True,
                        )
                # gru2 tail for step T-1
                t2b = tmp.tile([128, BL], F16, tag="t2b", name="t2b")
                emit_rh2(T - 1)
                emit_gru2_elt_a(T - 1, t2b)
                emit_gru2_elt_b(T - 1, t2b)

                # ---- dense tail ----
                pd = ps_zr1[T % 2]
                h2f = sb_h2[(T - 1) % 2]
                q3 = tmp.tile([64, 32], F16, tag="q3", name="q3")
                q4 = tmp.tile([32, 32], F16, tag="q4", name="q4")
                q5 = tmp.tile([32, 32], F32, tag="q5", name="q5")
                qt = tmp.tile([32, 32], F32, tag="qt", name="qt")
                nc.vector.memset(q5[:], 0.0)
                nc.tensor.matmul(pd[0:64, 0:32], sb_wd3[:], h2f[:], start=True, stop=True)
                nc.scalar.activation(
                    q3[:], pd[0:64, 0:32], AF.Identity, bias=sb_vbd[0:64, 0:1]
                )
                nc.tensor.matmul(pd[0:32, 32:64], sb_wd4[:], q3[:], start=False, stop=True)
                nc.scalar.activation(
                    q4[:], pd[0:32, 32:64], AF.Identity, bias=sb_vbd[0:32, 1:2]
                )
                nc.tensor.matmul(pd[0:OUT, 64:96], sb_wd5[:], q4[:], start=False, stop=True)
                nc.scalar.activation(
                    q5[0:OUT, :], pd[0:OUT, 64:96], AF.Identity, bias=sb_vbd[0:OUT, 2:3]
                )
                nc.vector.transpose(qt[:], q5[:])
                nc.sync.dma_start(out=d_y[:], in_=qt[0:BL, 0:OUT])

    nc.compile()
    return nc


def _run(inputs, T):
    in_maps, flags = _prep(inputs, T)
    nc = _build(T, flags)
    res = run_bass_kernel_spmd(nc, in_maps, core_ids=list(range(NCORES)))
    return np.concatenate([res.results[c]["y"] for c in range(NCORES)], 0).astype(
        np.float32
    )


def kernel(**inputs):
    return _run(inputs, T_FULL)


if __name__ == "__main__":
    rng = np.random.default_rng(0)
    ins = {
        "x": rng.standard_normal((B, T_FULL, F), np.float32),
        "k1": rng.standard_normal((F, 3 * U1), np.float32) * 0.05,
        "r1": rng.standard_normal((U1, 3 * U1), np.float32) * 0.05,
        "b1": np.zeros((2, 3 * U1), np.float32),
        "k2": rng.standard_normal((U1, 3 * U2), np.float32) * 0.05,
        "r2": rng.standard_normal((U2, 3 * U2), np.float32) * 0.05,
        "b2": np.zeros((2, 3 * U2), np.float32),
        "w3": rng.standard_normal((U2, 64), np.float32) * 0.05,
        "b3": np.zeros((64,), np.float32),
        "w4": rng.standard_normal((64, 32), np.float32) * 0.05,
        "b4": np.zeros((32,), np.float32),
        "w5": rng.standard_normal((32, OUT), np.float32) * 0.05,
        "b5": np.zeros((OUT,), np.float32),
    }
    y = _run(ins, 8)
    print("ran", y.shape, y[:2, :4])
